# revision 6
# baseline (speedup 1.0000x reference)
"""Butterworth bandpass filter (order-8 IIR, 9-tap b/a) over x[16, 64, 65536].

Strategy: the IIR is computed as a 128-tap causal FIR (tail l2 4.8e-3,
combining to ~7e-3 total rel err vs the 2e-2 gate) mapped onto the
TensorEngine as a banded block-Toeplitz matmul, with ALL device I/O in
bfloat16 to halve HBM traffic (the kernel is memory-bound):

  - 1024 signals sharded 128-per-core across 8 NeuronCores (data parallel).
  - x is cast to bf16 on the host; the device reads 16 MiB and writes
    16 MiB per core (32 MiB round trip -> ~93 us at the ~360 GB/s DMA
    roofline, vs 64 MiB / ~186 us for the f32 version).
  - Per core, x[128, 65536] bf16 is processed in output windows of 512.
    Input blocks [128 sig, 128 t] are PE-transposed (1 cyc/row for bf16)
    to [t, sig]; each window's y[sig, 512] accumulates 5 banded-Toeplitz
    matmuls (bf16 runs 1 cyc/row at ANY width, so slab widths are the
    natural 127/255/255/255/128 spans -- 1020 cols/window vs 1792 for the
    f32r version which needed >=256-wide slabs).
  - PE work: 1020 mm + 512 transpose cols/window = ~196k cycles ~ 82 us
    at 2.4 GHz, just under the DMA floor.
  - Transposed-input PSUM->SBUF copies run on the DVE (bf16->bf16 hits the
    2x_1p DVE mode: ~50 us); output PSUM f32 -> bf16 staging copies run on
    the otherwise-idle Activation engine (~73 us), keeping every engine
    under the DMA roofline.
  - Window J+1's transposes are interleaved between window J's matmuls.
  - Host casts the bf16 result back to f32.
"""

import os
from contextlib import ExitStack

import numpy as np

B, C, T = 16, 64, 65536
NSIG = B * C              # 1024 signals
N_CORES = 8
SIG_PER_CORE = NSIG // N_CORES  # 128

W = 128                   # FIR taps (tail l2 4.8e-3; total err ~7e-3 < 2e-2)
WIN = 512                 # output window (one PSUM bank of f32)
NWIN = T // WIN           # 128
CHUNK = 4096              # input DMA chunk (1 MiB bf16)
NCHUNK = T // CHUNK       # 16
WPC = CHUNK // WIN        # windows per input chunk = 8
OUT_CHUNK = 4096          # output DMA chunk (1 MiB bf16)
WPO = OUT_CHUNK // WIN    # windows per output chunk = 8

# Contributor p uses input block q = 4J - 1 + p; it covers window-local
# output cols [c0, c0+w) with slab_p[i, n] = h[n - 128(p-1) - i].
# bf16 matmuls run 1 cyc/row at any width, so widths are the natural spans.
SLAB_SPECS = [(0, 127), (0, 255), (128, 255), (256, 255), (384, 128)]
SLAB_OFFS = np.cumsum([0] + [w for _, w in SLAB_SPECS]).tolist()
SLAB_COLS = SLAB_OFFS[-1]  # 1020
# p=1 ([0,255)) runs first with start=True (clears the PSUM bank's
# has_written zero-region, as in the proven f32r version); the rest
# accumulate with start=False, last one carries stop=True.
EXEC_ORDER = [1, 0, 2, 3, 4]

_NC_CACHE = {}


def _build_nc():
    import concourse.bacc as bacc
    import concourse.tile as tile
    from concourse import mybir

    bf16 = mybir.dt.bfloat16
    f32 = mybir.dt.float32

    nc = bacc.Bacc("TRN2", target_bir_lowering=False, debug=False)
    x_d = nc.dram_tensor("x", [SIG_PER_CORE, T], bf16, kind="ExternalInput")
    slab_d = nc.dram_tensor("slabs", [128, SLAB_COLS], bf16, kind="ExternalInput")
    ident_d = nc.dram_tensor("ident", [128, 128], bf16, kind="ExternalInput")
    y_d = nc.dram_tensor("y", [SIG_PER_CORE, T], bf16, kind="ExternalOutput")

    with tile.TileContext(nc) as tc, ExitStack() as ctx:
        const = ctx.enter_context(tc.tile_pool(name="const", bufs=1))
        inpool = ctx.enter_context(tc.tile_pool(name="inpool", bufs=4))
        xtpool = ctx.enter_context(tc.tile_pool(name="xtpool", bufs=4))
        outpool = ctx.enter_context(tc.tile_pool(name="outpool", bufs=3))
        pst = ctx.enter_context(tc.tile_pool(name="pst", bufs=4, space="PSUM"))
        psy = ctx.enter_context(tc.tile_pool(name="psy", bufs=4, space="PSUM"))

        in_tiles = {}
        # Constants arrive pre-cast to bf16 from the host on the Act HWDGE
        # queue (shared with output DMAs), keeping the SP queue free for the
        # input stream.
        ident = const.tile([128, 128], bf16)
        nc.scalar.dma_start(ident[:], ident_d.ap()[:])
        slab = const.tile([128, SLAB_COLS], bf16)
        nc.scalar.dma_start(slab[:], slab_d.ap()[:])

        def load_chunk(g):
            # Input stream rides the SP HWDGE queue: SWDGE (gpsimd) inserts
            # periodic DRAIN barriers that serialize descriptor generation
            # and starved the PE for ~7 us at the head of the run.
            if g in in_tiles or g >= NCHUNK:
                return
            t_in = inpool.tile([SIG_PER_CORE, CHUNK], bf16, tag="in")
            if g == 0:
                # Taper the first chunk so window 0's transposes can start
                # after ~0.5 us instead of waiting for a full 1 MiB chunk.
                for lo, hi in ((0, 512), (512, 1024), (1024, 2048), (2048, CHUNK)):
                    nc.sync.dma_start(t_in[:, lo:hi], x_d.ap()[:, lo:hi])
            else:
                nc.sync.dma_start(t_in[:], x_d.ap()[:, g * CHUNK:(g + 1) * CHUNK])
            in_tiles[g] = t_in

        # Prefetch the first two chunks before anything else.
        load_chunk(0)
        load_chunk(1)

        xt_chunks = {}
        out_tile = None

        def emit_transposes(J):
            # Transpose the 4 input blocks of window J: [sig, t] -> [t, sig].
            # Returns the 4 transpose emitters so they can be interleaved
            # between the previous window's matmuls.
            g = J // WPC
            load_chunk(g + 1)
            xin, base = in_tiles[g], (J % WPC) * WIN
            ps_tr = pst.tile([128, 512], bf16, tag="ps_tr")

            def one(c):
                nc.tensor.transpose(
                    ps_tr[:, c * 128:(c + 1) * 128],
                    xin[:, base + c * 128: base + (c + 1) * 128],
                    ident[:],
                )

            def finish():
                xt = xtpool.tile([128, 512], bf16, tag="xt")
                nc.vector.tensor_copy(xt[:], ps_tr[:])
                xt_chunks[J] = xt
                if J - 2 in xt_chunks:
                    del xt_chunks[J - 2]
                if J // WPC - 2 in in_tiles and (J % WPC) == WPC - 1:
                    del in_tiles[J // WPC - 2]

            return one, finish

        one0, finish0 = emit_transposes(0)
        for c in range(4):
            one0(c)
        finish0()

        for J in range(NWIN):
            nxt = emit_transposes(J + 1) if J + 1 < NWIN else None

            # FIR window J: accumulate 5 banded-Toeplitz matmuls into one
            # PSUM bank, with window J+1's transposes interleaved between.
            ps_y = psy.tile([128, WIN], f32, tag="ps_y")
            live = [p for p in EXEC_ORDER if 4 * J - 1 + p >= 0]
            for k, p in enumerate(live):
                q = 4 * J - 1 + p
                if p == 0:
                    lhsT = xt_chunks[J - 1][:, 3 * 128:4 * 128]
                else:
                    lhsT = xt_chunks[J][:, (p - 1) * 128:p * 128]
                c0, w = SLAB_SPECS[p]
                off = SLAB_OFFS[p]
                nc.tensor.matmul(
                    ps_y[:, c0:c0 + w],
                    lhsT,
                    slab[:, off:off + w],
                    start=(k == 0),
                    stop=(k == len(live) - 1),
                )
                if nxt is not None and k < 4:
                    nxt[0](k)

            if J % WPO == 0:
                out_tile = outpool.tile([SIG_PER_CORE, OUT_CHUNK], bf16, tag="out")
            out_slice = out_tile[:, (J % WPO) * WIN:(J % WPO + 1) * WIN]
            # PSUM f32 -> SBUF bf16 cast-copies run on the Activation
            # engine, with every 4th window offloaded to the DVE so neither
            # engine approaches the DMA roofline.
            if J % 4 == 3:
                nc.vector.tensor_copy(out_slice, ps_y[:])
            else:
                nc.scalar.copy(out_slice, ps_y[:])
            if J // WPO == NWIN // WPO - 1:
                # Tail: ship each window as soon as its copy lands so the
                # final DMA drains ~0.4 us after the last compute, not 3 us.
                nc.scalar.dma_start(
                    y_d.ap()[:, J * WIN:(J + 1) * WIN],
                    out_slice,
                )
            elif J % WPO == WPO - 1:
                nc.scalar.dma_start(
                    y_d.ap()[:, (J // WPO) * OUT_CHUNK:(J // WPO + 1) * OUT_CHUNK],
                    out_tile[:],
                )
            if nxt is not None:
                nxt[1]()
    nc.compile()
    return nc


def _get_nc():
    if "nc" not in _NC_CACHE:
        _NC_CACHE["nc"] = _build_nc()
    return _NC_CACHE["nc"]


def _impulse_response(b, a, n):
    b = np.asarray(b, np.float64)
    a = np.asarray(a, np.float64)
    b = b / a[0]
    a = a / a[0]
    h = np.zeros(n, np.float64)
    for t in range(n):
        acc = b[t] if t < len(b) else 0.0
        kmax = min(len(a) - 1, t)
        for k in range(1, kmax + 1):
            acc -= a[k] * h[t - k]
        h[t] = acc
    return h


def _build_slabs(h):
    """slab_p[i, n] = h[n - 128 (p-1) - i] for n in [c0_p, c0_p+w_p)."""
    i = np.arange(128)
    slabs = np.zeros((128, SLAB_COLS), np.float32)
    for p, ((c0, w), off) in enumerate(zip(SLAB_SPECS, SLAB_OFFS)):
        n = c0 + np.arange(w)
        d = n[None, :] - 128 * (p - 1) - i[:, None]
        valid = (d >= 0) & (d < W)
        vals = np.where(valid, h[np.clip(d, 0, W - 1)], 0.0)
        slabs[:, off:off + w] = vals.astype(np.float32)
    return slabs


def kernel_with_results(x, b, a, trace=False):
    import ml_dtypes
    from concourse.bass_utils import run_bass_kernel_spmd

    bf16 = ml_dtypes.bfloat16
    h = _impulse_response(np.asarray(b), np.asarray(a), W)
    slabs = _build_slabs(h).astype(bf16)
    ident = np.eye(128, dtype=bf16)

    xs = np.asarray(x, np.float32).reshape(NSIG, T).astype(bf16)
    in_maps = [
        {
            "x": np.ascontiguousarray(xs[c * SIG_PER_CORE:(c + 1) * SIG_PER_CORE]),
            "slabs": slabs,
            "ident": ident,
        }
        for c in range(N_CORES)
    ]
    nc = _get_nc()
    res = run_bass_kernel_spmd(nc, in_maps, core_ids=list(range(N_CORES)), trace=trace)
    y = np.concatenate([res.results[c]["y"] for c in range(N_CORES)], axis=0)
    return y.astype(np.float32).reshape(B, C, T), res


def kernel(x, b, a):
    os.environ.setdefault("BASS_NEVER_TRACE", "1")
    y, _ = kernel_with_results(x, b, a, trace=False)
    return y
